# revision 2
# baseline (speedup 1.0000x reference)
"""Trainium2 Bass kernel for nn_Decoder (GNN message passing decoder), v2.

Reference computation:
    v1 = z_out + z_self                         # [N, C]
    v2 = z_in + z_self                          # [N, C]
    value = v1[src] * v2[dst]                   # [E, C]
    h = elu(value @ W1 + b1)                    # [E, H]
    out = sigmoid(h @ W2 + b2)                  # [E, 1]

Strategy (8 NeuronCores, SPMD, edge-sharded):
  - Edges split into 8 contiguous slices of E/8; node tables and MLP weights
    replicated.
  - Host precomputes the fp16 node tables v1 = z_out + z_self and
    v2 = z_in + z_self ([N, C], 256B rows).  Each edge then needs exactly two
    256B row gathers (half the bytes of the previous 2x1KB pair-table design).
  - Gathers use the SWDGE dma_gather ucode with transpose=True: fp16 rows land
    channel-on-partition ([128, cap] tiles, edges on the free dim), so the MLP
    runs directly on the gathered tiles - no PE transposes, no PSUM copies.
  - int16 index requirement is handled by splitting the node table into 4
    chunks of 25000 rows and bucketing each core's edges by
    (src_chunk, dst_chunk) on the host (16 buckets, one gather op per bucket
    per table, padded to a common capacity `cap`).  The bucket permutation is
    undone on the host at the end.
  - On-chip per 1024-edge stack: mm1 with stationary lhsT [W1 | -W1] produces
    [s; -s] in PSUM; one ACT Relu pass (bias [b1; -b1]) gives
    [relu(s); relu(-s)] stacked; one ACT Exp pass (in-place, scale=-1) turns
    the bottom half into exp(min(s,0)).  mm2 runs TRANSPOSED per 128-edge
    subtile: lhsT = hes[:, e:e+128], rhs = [W2/2; W2/2] -> out [128 edges, 1],
    so logits pack edge-on-partition into a [128, 512] PSUM tile (free-dim
    column per subtile).  The final sigmoid-via-tanh ACT pass and the
    0.5*x+0.5 DVE pass then run once per 65536 edges.  elu's -1 is folded
    into the output bias: b2' = b2 - sum(W2).
"""
import sys

if "/opt/trn_rl_repo" not in sys.path:
    sys.path.insert(0, "/opt/trn_rl_repo")

import math

import numpy as np

N, C, E, H = 100000, 128, 600000, 64
M = 8                    # cores
EPC = E // M             # edges per core
NCHUNK = 4               # node-table chunks (rows fit in int16)
CHUNK = N // NCHUNK      # 25000
NB = NCHUNK * NCHUNK     # (src_chunk, dst_chunk) buckets
SW = 1024                # mm1/ELU stack width (PSUM tile cols)
OW = 512                 # output group width (mm2 slab / out row)

_BUILD_CACHE: dict = {}


def _build(caps, b2p: float, *, do_gather: bool = True,
           do_compute: bool = True, nqueues: int = 1, gop: int = 0,
           gop_tail: int = 0, ntail: int = 2,
           gat_bufs: int = 2, vec_bufs: int = 2, stack_bufs: int = 3,
           psum_bufs: int = 2, psumo_bufs: int = 2, idx_bufs: int = 3):
    from concourse import bacc, mybir
    import concourse.tile as tile

    f32 = mybir.dt.float32
    f16 = mybir.dt.float16
    i16 = mybir.dt.int16
    AF = mybir.ActivationFunctionType
    OP = mybir.AluOpType

    if isinstance(caps, int):
        caps = (caps,) * NB
    caps = tuple(int(c) for c in caps)
    assert len(caps) == NB and all(c % 128 == 0 for c in caps)
    total = sum(caps)
    prefix = [0]
    for c in caps:
        prefix.append(prefix[-1] + c)
    if gop <= 0:
        gop = max(caps)          # gather op granularity (indices per SWDGE op)
    if gop_tail <= 0:
        gop_tail = gop           # finer granularity for the last buckets
    assert gop % 128 == 0 and gop_tail % 128 == 0
    ncols = total // 16
    ntiles = total // 128        # 128-edge subtiles per core
    nfills = (ntiles + OW - 1) // OW

    nc = bacc.Bacc("TRN2", target_bir_lowering=False, num_swdge_queues=nqueues)
    v1tab = nc.dram_tensor("v1tab", [N, C], f16, kind="ExternalInput")
    v2tab = nc.dram_tensor("v2tab", [N, C], f16, kind="ExternalInput")
    isrc = nc.dram_tensor("isrc", [128, ncols], i16, kind="ExternalInput")
    idst = nc.dram_tensor("idst", [128, ncols], i16, kind="ExternalInput")
    # stacked [W1 | -W1]: one mm1 emits [s; -s]; one Relu pass with bias
    # [b1; -b1] then yields [relu(s); relu(-s)] in a single [128, *] tile
    w1 = nc.dram_tensor("w1", [C, 2 * H], f16, kind="ExternalInput")
    b1 = nc.dram_tensor("b1", [2 * H, 1], f32, kind="ExternalInput")
    # stacked [W2/2 ; W2/2]: one mm2 contracts the relu-part (rows 0:64)
    # and the exp-part (rows 64:128) of the stacked activation tile at once
    w2 = nc.dram_tensor("w2", [2 * H, 1], f16, kind="ExternalInput")
    # out[f, p, k]: edge (f*OW + k)*128 + p of this core's padded stream
    out = nc.dram_tensor("out", [nfills, 128, OW], f32, kind="ExternalOutput")

    with tile.TileContext(nc) as tc:
        with (
            tc.tile_pool(name="const", bufs=1) as constp,
            tc.tile_pool(name="idx", bufs=idx_bufs) as idxp,
            tc.tile_pool(name="gat", bufs=gat_bufs) as gat,
            tc.tile_pool(name="vec", bufs=vec_bufs) as vec,
            tc.tile_pool(name="stack", bufs=stack_bufs) as stackp,
            tc.tile_pool(name="ostp", bufs=2) as ostp,
            tc.tile_pool(name="small", bufs=2) as small,
            tc.tile_pool(name="psum", bufs=psum_bufs, space="PSUM") as psum,
            tc.tile_pool(name="psumo", bufs=psumo_bufs, space="PSUM") as psumo,
        ):
            w1t = constp.tile([C, 2 * H], f16)
            nc.sync.dma_start(out=w1t[:], in_=w1[:, :])
            b1t = constp.tile([2 * H, 1], f32)
            nc.sync.dma_start(out=b1t[:], in_=b1[:, :])
            w2t = constp.tile([2 * H, 1], f16)
            nc.sync.dma_start(out=w2t[:], in_=w2[:, :])
            b2ht = constp.tile([128, 1], f32)
            nc.vector.memset(b2ht[:], float(b2p) * 0.5)

            ti = 0               # global 128-edge subtile counter
            lg = None
            for b in range(NB):
                cap = caps[b]
                icols = cap // 16
                sc, dc = divmod(b, NCHUNK)
                col0 = prefix[b] // 16
                qn = b % nqueues
                isl = idxp.tile([128, icols], i16, tag="isl")
                nc.sync.dma_start(out=isl[:], in_=isrc[:, col0:col0 + icols])
                idl = idxp.tile([128, icols], i16, tag="idl")
                nc.sync.dma_start(out=idl[:], in_=idst[:, col0:col0 + icols])
                v1g = gat.tile([128, 1, cap], f16, tag="v1g")
                v2g = gat.tile([128, 1, cap], f16, tag="v2g")
                val = vec.tile([128, cap], f16, tag="val")
                bgop = gop_tail if b >= NB - ntail else gop
                for g0 in range(0, cap, bgop):
                    gw = min(bgop, cap - g0)
                    if do_gather:
                        gc0 = g0 // 16
                        gcols = gw // 16
                        nc.gpsimd.dma_gather(
                            out_ap=v1g[:, :, g0:g0 + gw],
                            in_ap=v1tab[sc * CHUNK:(sc + 1) * CHUNK, :],
                            idxs_ap=isl[:, gc0:gc0 + gcols],
                            num_idxs=gw, num_idxs_reg=gw, elem_size=C,
                            transpose=True, single_packet=False, queue_num=qn,
                        )
                        nc.gpsimd.dma_gather(
                            out_ap=v2g[:, :, g0:g0 + gw],
                            in_ap=v2tab[dc * CHUNK:(dc + 1) * CHUNK, :],
                            idxs_ap=idl[:, gc0:gc0 + gcols],
                            num_idxs=gw, num_idxs_reg=gw, elem_size=C,
                            transpose=True, single_packet=False, queue_num=qn,
                        )
                    else:
                        nc.vector.memset(v1g[:, 0, g0:g0 + 16], 0.5)
                        nc.vector.memset(v2g[:, 0, g0:g0 + 16], 0.5)
                    if do_compute:
                        # per-slice product: compute starts as soon as a
                        # gather slice lands instead of waiting for the bucket
                        nc.vector.tensor_tensor(
                            out=val[:, g0:g0 + gw],
                            in0=v1g[:, 0, g0:g0 + gw],
                            in1=v2g[:, 0, g0:g0 + gw], op=OP.mult)
                if not do_compute:
                    sink = small.tile([128, 16], f32, tag="sink")
                    nc.vector.tensor_tensor(
                        out=sink[:], in0=v1g[:, 0, 0:16], in1=v2g[:, 0, 0:16],
                        op=OP.add)
                    ti += cap // 128
                    continue
                s0 = 0
                while s0 < cap:
                    w = SW if cap - s0 >= SW else cap - s0
                    spf = psum.tile([128, SW], f32, tag="sp")
                    sp = spf[:, 0:w]
                    m0 = 0
                    while m0 < w:
                        mw = min(512, w - m0)
                        nc.tensor.matmul(
                            out=sp[:, m0:m0 + mw], lhsT=w1t[:],
                            rhs=val[:, s0 + m0:s0 + m0 + mw],
                            start=True, stop=True)
                        m0 += mw
                    hesf = stackp.tile([128, SW], f16, tag="hes")
                    hes = hesf[:, 0:w]
                    nc.scalar.activation(
                        out=hes[:, :], in_=sp[:, :], func=AF.Relu, bias=b1t[:])
                    # bottom half: relu(-s-b1) -> exp(-(that)) = exp(min(s+b1,0))
                    nc.scalar.activation(
                        out=hes[H:2 * H, :], in_=hes[H:2 * H, :], func=AF.Exp,
                        scale=-1.0)
                    for q in range(w // 128):
                        k = ti % OW
                        if k == 0:
                            lg = psumo.tile([128, OW], f32, tag="lg")
                            f0 = ti // OW
                        # transposed mm2: contract the 128 stacked h-rows of a
                        # 128-edge subtile -> [128 edges, 1] logit column
                        nc.tensor.matmul(
                            out=lg[:, k:k + 1],
                            lhsT=hes[:, q * 128:(q + 1) * 128], rhs=w2t[:],
                            start=True, stop=True)
                        if k == OW - 1 or ti == ntiles - 1:
                            cols = k + 1
                            ot = ostp.tile([128, OW], f32, tag="ot")
                            # sigmoid(x) = 0.5*tanh(x/2) + 0.5; w2 is
                            # pre-halved, so lg = logits/2. Tanh shares the
                            # ACT table set with Relu/Exp -> no table reloads.
                            nc.scalar.activation(
                                out=ot[:, 0:cols], in_=lg[:, 0:cols],
                                func=AF.Tanh, bias=b2ht[:])
                            nc.vector.tensor_scalar(
                                out=ot[:, 0:cols], in0=ot[:, 0:cols],
                                scalar1=0.5, scalar2=0.5,
                                op0=OP.mult, op1=OP.add)
                            nc.sync.dma_start(
                                out=out[f0, :, 0:cols], in_=ot[:, 0:cols])
                        ti += 1
                    s0 += w
    nc.compile()
    return nc


def _wrap_idxs(arr: np.ndarray) -> np.ndarray:
    """[total] int16 -> [128, total//16], 16-partition wrapped, replicated 8x
    across partition groups.  (Uniform 16-wrap: any contiguous column slice
    unwraps to the matching contiguous flat range, so per-bucket/per-op
    slices need no special blocking.)"""
    w16 = arr.reshape(-1, 16).T
    return np.ascontiguousarray(np.tile(w16, (8, 1)))


def _prep(edge_index: np.ndarray):
    """Bucket each core's edge slice by (src_chunk, dst_chunk).

    Returns caps (per-bucket capacities) and per-core
    (isrc, idst, flat_pos, orig_ids)."""
    src = edge_index[0].astype(np.int64)
    dst = edge_index[1].astype(np.int64)

    per_core = []
    bucket_max = np.zeros(NB, dtype=np.int64)
    for c in range(M):
        s = src[c * EPC:(c + 1) * EPC]
        d = dst[c * EPC:(c + 1) * EPC]
        bkt = (s // CHUNK) * NCHUNK + (d // CHUNK)
        order = np.argsort(bkt, kind="stable")
        counts = np.bincount(bkt, minlength=NB).astype(np.int64)
        bucket_max = np.maximum(bucket_max, counts)
        per_core.append((s, d, order, counts))

    caps = tuple(int(128 * max(1, math.ceil(m / 128))) for m in bucket_max)
    total = sum(caps)
    prefix = np.concatenate([[0], np.cumsum(caps)])

    prepped = []
    for c in range(M):
        s, d, order, counts = per_core[c]
        isrc_flat = np.zeros(total, dtype=np.int16)
        idst_flat = np.zeros(total, dtype=np.int16)
        flat_pos = np.empty(EPC, dtype=np.int64)
        orig_ids = np.empty(EPC, dtype=np.int64)
        ofs = 0
        w = 0
        for b in range(NB):
            k = int(counts[b])
            p0 = int(prefix[b])
            sel = order[ofs:ofs + k]
            sc, dc = divmod(b, NCHUNK)
            isrc_flat[p0:p0 + k] = (s[sel] - sc * CHUNK).astype(np.int16)
            idst_flat[p0:p0 + k] = (d[sel] - dc * CHUNK).astype(np.int16)
            flat_pos[w:w + k] = p0 + np.arange(k)
            orig_ids[w:w + k] = c * EPC + sel
            ofs += k
            w += k
        assert w == EPC
        prepped.append((
            _wrap_idxs(isrc_flat),
            _wrap_idxs(idst_flat),
            flat_pos,
            orig_ids,
        ))
    return caps, prepped


def prepare(z_in, z_out, z_self, edge_index, W1, b1, W2, b2):
    """Host-side prep: fp16 node tables, bucketed int16 indices, stacked
    weight layouts.  Returns (cap, b2p, prepped, in_maps)."""
    z_in = np.asarray(z_in, dtype=np.float32)
    z_out = np.asarray(z_out, dtype=np.float32)
    z_self = np.asarray(z_self, dtype=np.float32)
    edge_index = np.asarray(edge_index)
    W1 = np.asarray(W1, dtype=np.float32)
    b1 = np.asarray(b1, dtype=np.float32)
    W2 = np.asarray(W2, dtype=np.float32)
    b2 = np.asarray(b2, dtype=np.float32)

    v1tab = np.ascontiguousarray((z_out + z_self).astype(np.float16))
    v2tab = np.ascontiguousarray((z_in + z_self).astype(np.float16))
    b2p = float(b2.reshape(-1)[0] - W2.sum())

    caps, prepped = _prep(edge_index)

    w1m = np.ascontiguousarray(
        np.concatenate([W1, -W1], axis=1).astype(np.float16))    # [C, 2H]
    b1m = np.ascontiguousarray(
        np.concatenate([b1, -b1]).reshape(2 * H, 1).astype(np.float32))
    # halved (sigmoid via 0.5*tanh(logits/2)+0.5) and stacked twice for the
    # [relu-part ; exp-part] stacked mm2 contraction
    w2h = W2.reshape(H, 1) * 0.5
    w2m = np.ascontiguousarray(
        np.concatenate([w2h, w2h], axis=0).astype(np.float16))

    in_maps = []
    for c in range(M):
        isrc_c, idst_c, _, _ = prepped[c]
        in_maps.append({
            "v1tab": v1tab, "v2tab": v2tab,
            "isrc": isrc_c, "idst": idst_c,
            "w1": w1m, "b1": b1m, "w2": w2m,
        })
    return caps, b2p, prepped, in_maps


def _run(z_in, z_out, z_self, edge_index, W1, b1, W2, b2, **spmd_kwargs):
    from concourse.bass_utils import run_bass_kernel_spmd

    caps, b2p, prepped, in_maps = prepare(
        z_in, z_out, z_self, edge_index, W1, b1, W2, b2)

    key = (caps, round(b2p, 10))
    if key not in _BUILD_CACHE:
        _BUILD_CACHE.clear()
        _BUILD_CACHE[key] = _build(caps, b2p, gop=2560, gop_tail=1024)
    nc = _BUILD_CACHE[key]

    res = run_bass_kernel_spmd(nc, in_maps, core_ids=list(range(M)), **spmd_kwargs)

    out_full = np.zeros(E, dtype=np.float32)
    for c in range(M):
        _, _, flat_pos, orig_ids = prepped[c]
        # out[f, p, k] holds edge (f*OW + k)*128 + p of the padded stream
        core_flat = res.results[c]["out"].transpose(0, 2, 1).reshape(-1)
        out_full[orig_ids] = core_flat[flat_pos]
    return out_full.reshape(E, 1), res


def kernel(z_in, z_out, z_self, edge_index, W1, b1, W2, b2):
    out, _ = _run(z_in, z_out, z_self, edge_index, W1, b1, W2, b2)
    return out


# revision 3
# speedup vs baseline: 1.0078x; 1.0078x over previous
"""Trainium2 Bass kernel for nn_Decoder (GNN message passing decoder), v2.

Reference computation:
    v1 = z_out + z_self                         # [N, C]
    v2 = z_in + z_self                          # [N, C]
    value = v1[src] * v2[dst]                   # [E, C]
    h = elu(value @ W1 + b1)                    # [E, H]
    out = sigmoid(h @ W2 + b2)                  # [E, 1]

Strategy (8 NeuronCores, SPMD, edge-sharded):
  - Edges split into 8 contiguous slices of E/8; node tables and MLP weights
    replicated.
  - Host precomputes the fp16 node tables v1 = z_out + z_self and
    v2 = z_in + z_self ([N, C], 256B rows).  Each edge then needs exactly two
    256B row gathers (half the bytes of the previous 2x1KB pair-table design).
  - Gathers use the SWDGE dma_gather ucode with transpose=True: fp16 rows land
    channel-on-partition ([128, cap] tiles, edges on the free dim), so the MLP
    runs directly on the gathered tiles - no PE transposes, no PSUM copies.
  - int16 index requirement is handled by splitting the node table into 4
    chunks of 25000 rows and bucketing each core's edges by
    (src_chunk, dst_chunk) on the host (16 buckets, one gather op per bucket
    per table, padded to a common capacity `cap`).  The bucket permutation is
    undone on the host at the end.
  - On-chip per 1024-edge stack: mm1 with stationary lhsT [W1 | -W1] produces
    [s; -s] in PSUM; one ACT Relu pass (bias [b1; -b1]) gives
    [relu(s); relu(-s)] stacked; one ACT Exp pass (in-place, scale=-1) turns
    the bottom half into exp(min(s,0)).  mm2 runs TRANSPOSED per 128-edge
    subtile: lhsT = hes[:, e:e+128], rhs = [W2/2; W2/2] -> out [128 edges, 1],
    so logits pack edge-on-partition into a [128, 512] PSUM tile (free-dim
    column per subtile).  The final sigmoid-via-tanh ACT pass and the
    0.5*x+0.5 DVE pass then run once per 65536 edges.  elu's -1 is folded
    into the output bias: b2' = b2 - sum(W2).
"""
import sys

if "/opt/trn_rl_repo" not in sys.path:
    sys.path.insert(0, "/opt/trn_rl_repo")

import math

import numpy as np

N, C, E, H = 100000, 128, 600000, 64
M = 8                    # cores
EPC = E // M             # edges per core
NCHUNK = 4               # node-table chunks (rows fit in int16)
CHUNK = N // NCHUNK      # 25000
NB = NCHUNK * NCHUNK     # (src_chunk, dst_chunk) buckets
SW = 1024                # mm1/ELU stack width (PSUM tile cols)
OW = 512                 # output group width (mm2 slab / out row)

_BUILD_CACHE: dict = {}


def _build(caps, b2p: float, *, do_gather: bool = True,
           do_compute: bool = True, nqueues: int = 1, gop: int = 0,
           gop_tail: int = 0, ntail: int = 2,
           gat_bufs: int = 2, vec_bufs: int = 2, stack_bufs: int = 3,
           psum_bufs: int = 2, psumo_bufs: int = 2, idx_bufs: int = 3):
    from concourse import bacc, mybir
    import concourse.tile as tile

    f32 = mybir.dt.float32
    f16 = mybir.dt.float16
    i16 = mybir.dt.int16
    AF = mybir.ActivationFunctionType
    OP = mybir.AluOpType

    if isinstance(caps, int):
        caps = (caps,) * NB
    caps = tuple(int(c) for c in caps)
    assert len(caps) == NB and all(c % 128 == 0 for c in caps)
    total = sum(caps)
    prefix = [0]
    for c in caps:
        prefix.append(prefix[-1] + c)
    if gop <= 0:
        gop = max(caps)          # gather op granularity (indices per SWDGE op)
    if gop_tail <= 0:
        gop_tail = gop           # finer granularity for the last buckets
    assert gop % 128 == 0 and gop_tail % 128 == 0
    ncols = total // 16
    ntiles = total // 128        # 128-edge subtiles per core
    nfills = (ntiles + OW - 1) // OW

    nc = bacc.Bacc("TRN2", target_bir_lowering=False, num_swdge_queues=nqueues)
    v1tab = nc.dram_tensor("v1tab", [N, C], f16, kind="ExternalInput")
    v2tab = nc.dram_tensor("v2tab", [N, C], f16, kind="ExternalInput")
    isrc = nc.dram_tensor("isrc", [128, ncols], i16, kind="ExternalInput")
    idst = nc.dram_tensor("idst", [128, ncols], i16, kind="ExternalInput")
    # stacked [W1 | -W1]: one mm1 emits [s; -s]; one Relu pass with bias
    # [b1; -b1] then yields [relu(s); relu(-s)] in a single [128, *] tile
    w1 = nc.dram_tensor("w1", [C, 2 * H], f16, kind="ExternalInput")
    b1 = nc.dram_tensor("b1", [2 * H, 1], f32, kind="ExternalInput")
    # stacked [W2/2 ; W2/2]: one mm2 contracts the relu-part (rows 0:64)
    # and the exp-part (rows 64:128) of the stacked activation tile at once
    w2 = nc.dram_tensor("w2", [2 * H, 1], f16, kind="ExternalInput")
    # out[f, p, k]: edge (f*OW + k)*128 + p of this core's padded stream
    out = nc.dram_tensor("out", [nfills, 128, OW], f32, kind="ExternalOutput")

    with tile.TileContext(nc) as tc:
        with (
            tc.tile_pool(name="const", bufs=1) as constp,
            tc.tile_pool(name="idx", bufs=idx_bufs) as idxp,
            tc.tile_pool(name="gat", bufs=gat_bufs) as gat,
            tc.tile_pool(name="vec", bufs=vec_bufs) as vec,
            tc.tile_pool(name="stack", bufs=stack_bufs) as stackp,
            tc.tile_pool(name="ostp", bufs=2) as ostp,
            tc.tile_pool(name="small", bufs=2) as small,
            tc.tile_pool(name="psum", bufs=psum_bufs, space="PSUM") as psum,
            tc.tile_pool(name="psumo", bufs=psumo_bufs, space="PSUM") as psumo,
        ):
            # bucket-0 idx slices first: they gate the first gather, so their
            # DMAs must precede the (latency-tolerant) const loads
            isl0 = idxp.tile([128, caps[0] // 16], i16, tag="isl")
            nc.sync.dma_start(out=isl0[:], in_=isrc[:, 0:caps[0] // 16])
            idl0 = idxp.tile([128, caps[0] // 16], i16, tag="idl")
            nc.sync.dma_start(out=idl0[:], in_=idst[:, 0:caps[0] // 16])
            w1t = constp.tile([C, 2 * H], f16)
            nc.sync.dma_start(out=w1t[:], in_=w1[:, :])
            b1t = constp.tile([2 * H, 1], f32)
            nc.sync.dma_start(out=b1t[:], in_=b1[:, :])
            w2t = constp.tile([2 * H, 1], f16)
            nc.sync.dma_start(out=w2t[:], in_=w2[:, :])
            b2ht = constp.tile([128, 1], f32)
            nc.vector.memset(b2ht[:], float(b2p) * 0.5)

            ti = 0               # global 128-edge subtile counter
            lg = None
            for b in range(NB):
                cap = caps[b]
                icols = cap // 16
                sc, dc = divmod(b, NCHUNK)
                col0 = prefix[b] // 16
                qn = b % nqueues
                if b == 0:
                    isl, idl = isl0, idl0
                else:
                    isl = idxp.tile([128, icols], i16, tag="isl")
                    nc.sync.dma_start(out=isl[:], in_=isrc[:, col0:col0 + icols])
                    idl = idxp.tile([128, icols], i16, tag="idl")
                    nc.sync.dma_start(out=idl[:], in_=idst[:, col0:col0 + icols])
                v1g = gat.tile([128, 1, cap], f16, tag="v1g")
                v2g = gat.tile([128, 1, cap], f16, tag="v2g")
                val = vec.tile([128, cap], f16, tag="val")
                bgop = gop_tail if (b >= NB - ntail or b == 0) else gop
                for g0 in range(0, cap, bgop):
                    gw = min(bgop, cap - g0)
                    if do_gather:
                        gc0 = g0 // 16
                        gcols = gw // 16
                        nc.gpsimd.dma_gather(
                            out_ap=v1g[:, :, g0:g0 + gw],
                            in_ap=v1tab[sc * CHUNK:(sc + 1) * CHUNK, :],
                            idxs_ap=isl[:, gc0:gc0 + gcols],
                            num_idxs=gw, num_idxs_reg=gw, elem_size=C,
                            transpose=True, single_packet=False, queue_num=qn,
                        )
                        nc.gpsimd.dma_gather(
                            out_ap=v2g[:, :, g0:g0 + gw],
                            in_ap=v2tab[dc * CHUNK:(dc + 1) * CHUNK, :],
                            idxs_ap=idl[:, gc0:gc0 + gcols],
                            num_idxs=gw, num_idxs_reg=gw, elem_size=C,
                            transpose=True, single_packet=False, queue_num=qn,
                        )
                    else:
                        nc.vector.memset(v1g[:, 0, g0:g0 + 16], 0.5)
                        nc.vector.memset(v2g[:, 0, g0:g0 + 16], 0.5)
                    if do_compute:
                        # per-slice product: compute starts as soon as a
                        # gather slice lands instead of waiting for the bucket
                        nc.vector.tensor_tensor(
                            out=val[:, g0:g0 + gw],
                            in0=v1g[:, 0, g0:g0 + gw],
                            in1=v2g[:, 0, g0:g0 + gw], op=OP.mult)
                if not do_compute:
                    sink = small.tile([128, 16], f32, tag="sink")
                    nc.vector.tensor_tensor(
                        out=sink[:], in0=v1g[:, 0, 0:16], in1=v2g[:, 0, 0:16],
                        op=OP.add)
                    ti += cap // 128
                    continue
                s0 = 0
                while s0 < cap:
                    w = SW if cap - s0 >= SW else cap - s0
                    spf = psum.tile([128, SW], f32, tag="sp")
                    sp = spf[:, 0:w]
                    m0 = 0
                    while m0 < w:
                        mw = min(512, w - m0)
                        nc.tensor.matmul(
                            out=sp[:, m0:m0 + mw], lhsT=w1t[:],
                            rhs=val[:, s0 + m0:s0 + m0 + mw],
                            start=True, stop=True)
                        m0 += mw
                    hesf = stackp.tile([128, SW], f16, tag="hes")
                    hes = hesf[:, 0:w]
                    nc.scalar.activation(
                        out=hes[:, :], in_=sp[:, :], func=AF.Relu, bias=b1t[:])
                    # bottom half: relu(-s-b1) -> exp(-(that)) = exp(min(s+b1,0))
                    nc.scalar.activation(
                        out=hes[H:2 * H, :], in_=hes[H:2 * H, :], func=AF.Exp,
                        scale=-1.0)
                    for q in range(w // 128):
                        k = ti % OW
                        if k == 0:
                            lg = psumo.tile([128, OW], f32, tag="lg")
                            f0 = ti // OW
                        # transposed mm2: contract the 128 stacked h-rows of a
                        # 128-edge subtile -> [128 edges, 1] logit column
                        nc.tensor.matmul(
                            out=lg[:, k:k + 1],
                            lhsT=hes[:, q * 128:(q + 1) * 128], rhs=w2t[:],
                            start=True, stop=True)
                        if k == OW - 1 or ti == ntiles - 1:
                            cols = k + 1
                            ot = ostp.tile([128, OW], f32, tag="ot")
                            # sigmoid(x) = 0.5*tanh(x/2) + 0.5; w2 is
                            # pre-halved, so lg = logits/2. Tanh shares the
                            # ACT table set with Relu/Exp -> no table reloads.
                            nc.scalar.activation(
                                out=ot[:, 0:cols], in_=lg[:, 0:cols],
                                func=AF.Tanh, bias=b2ht[:])
                            nc.vector.tensor_scalar(
                                out=ot[:, 0:cols], in0=ot[:, 0:cols],
                                scalar1=0.5, scalar2=0.5,
                                op0=OP.mult, op1=OP.add)
                            nc.sync.dma_start(
                                out=out[f0, :, 0:cols], in_=ot[:, 0:cols])
                        ti += 1
                    s0 += w
    nc.compile()
    return nc


def _wrap_idxs(arr: np.ndarray) -> np.ndarray:
    """[total] int16 -> [128, total//16], 16-partition wrapped, replicated 8x
    across partition groups.  (Uniform 16-wrap: any contiguous column slice
    unwraps to the matching contiguous flat range, so per-bucket/per-op
    slices need no special blocking.)"""
    w16 = arr.reshape(-1, 16).T
    return np.ascontiguousarray(np.tile(w16, (8, 1)))


def _prep(edge_index: np.ndarray):
    """Bucket each core's edge slice by (src_chunk, dst_chunk).

    Returns caps (per-bucket capacities) and per-core
    (isrc, idst, flat_pos, orig_ids)."""
    src = edge_index[0].astype(np.int64)
    dst = edge_index[1].astype(np.int64)

    per_core = []
    bucket_max = np.zeros(NB, dtype=np.int64)
    for c in range(M):
        s = src[c * EPC:(c + 1) * EPC]
        d = dst[c * EPC:(c + 1) * EPC]
        bkt = (s // CHUNK) * NCHUNK + (d // CHUNK)
        order = np.argsort(bkt, kind="stable")
        counts = np.bincount(bkt, minlength=NB).astype(np.int64)
        bucket_max = np.maximum(bucket_max, counts)
        per_core.append((s, d, order, counts))

    caps = tuple(int(128 * max(1, math.ceil(m / 128))) for m in bucket_max)
    total = sum(caps)
    prefix = np.concatenate([[0], np.cumsum(caps)])

    prepped = []
    for c in range(M):
        s, d, order, counts = per_core[c]
        isrc_flat = np.zeros(total, dtype=np.int16)
        idst_flat = np.zeros(total, dtype=np.int16)
        flat_pos = np.empty(EPC, dtype=np.int64)
        orig_ids = np.empty(EPC, dtype=np.int64)
        ofs = 0
        w = 0
        for b in range(NB):
            k = int(counts[b])
            p0 = int(prefix[b])
            sel = order[ofs:ofs + k]
            sc, dc = divmod(b, NCHUNK)
            isrc_flat[p0:p0 + k] = (s[sel] - sc * CHUNK).astype(np.int16)
            idst_flat[p0:p0 + k] = (d[sel] - dc * CHUNK).astype(np.int16)
            flat_pos[w:w + k] = p0 + np.arange(k)
            orig_ids[w:w + k] = c * EPC + sel
            ofs += k
            w += k
        assert w == EPC
        prepped.append((
            _wrap_idxs(isrc_flat),
            _wrap_idxs(idst_flat),
            flat_pos,
            orig_ids,
        ))
    return caps, prepped


def prepare(z_in, z_out, z_self, edge_index, W1, b1, W2, b2):
    """Host-side prep: fp16 node tables, bucketed int16 indices, stacked
    weight layouts.  Returns (cap, b2p, prepped, in_maps)."""
    z_in = np.asarray(z_in, dtype=np.float32)
    z_out = np.asarray(z_out, dtype=np.float32)
    z_self = np.asarray(z_self, dtype=np.float32)
    edge_index = np.asarray(edge_index)
    W1 = np.asarray(W1, dtype=np.float32)
    b1 = np.asarray(b1, dtype=np.float32)
    W2 = np.asarray(W2, dtype=np.float32)
    b2 = np.asarray(b2, dtype=np.float32)

    v1tab = np.ascontiguousarray((z_out + z_self).astype(np.float16))
    v2tab = np.ascontiguousarray((z_in + z_self).astype(np.float16))
    b2p = float(b2.reshape(-1)[0] - W2.sum())

    caps, prepped = _prep(edge_index)

    w1m = np.ascontiguousarray(
        np.concatenate([W1, -W1], axis=1).astype(np.float16))    # [C, 2H]
    b1m = np.ascontiguousarray(
        np.concatenate([b1, -b1]).reshape(2 * H, 1).astype(np.float32))
    # halved (sigmoid via 0.5*tanh(logits/2)+0.5) and stacked twice for the
    # [relu-part ; exp-part] stacked mm2 contraction
    w2h = W2.reshape(H, 1) * 0.5
    w2m = np.ascontiguousarray(
        np.concatenate([w2h, w2h], axis=0).astype(np.float16))

    in_maps = []
    for c in range(M):
        isrc_c, idst_c, _, _ = prepped[c]
        in_maps.append({
            "v1tab": v1tab, "v2tab": v2tab,
            "isrc": isrc_c, "idst": idst_c,
            "w1": w1m, "b1": b1m, "w2": w2m,
        })
    return caps, b2p, prepped, in_maps


def _run(z_in, z_out, z_self, edge_index, W1, b1, W2, b2, **spmd_kwargs):
    from concourse.bass_utils import run_bass_kernel_spmd

    caps, b2p, prepped, in_maps = prepare(
        z_in, z_out, z_self, edge_index, W1, b1, W2, b2)

    key = (caps, round(b2p, 10))
    if key not in _BUILD_CACHE:
        _BUILD_CACHE.clear()
        _BUILD_CACHE[key] = _build(caps, b2p, gop=2560, gop_tail=1024,
                                   psum_bufs=3, stack_bufs=4)
    nc = _BUILD_CACHE[key]

    res = run_bass_kernel_spmd(nc, in_maps, core_ids=list(range(M)), **spmd_kwargs)

    out_full = np.zeros(E, dtype=np.float32)
    for c in range(M):
        _, _, flat_pos, orig_ids = prepped[c]
        # out[f, p, k] holds edge (f*OW + k)*128 + p of the padded stream
        core_flat = res.results[c]["out"].transpose(0, 2, 1).reshape(-1)
        out_full[orig_ids] = core_flat[flat_pos]
    return out_full.reshape(E, 1), res


def kernel(z_in, z_out, z_self, edge_index, W1, b1, W2, b2):
    out, _ = _run(z_in, z_out, z_self, edge_index, W1, b1, W2, b2)
    return out
